# revision 33
# baseline (speedup 1.0000x reference)
"""Trainium2 Bass kernel for nn_BiLSTMNet (2-layer BiLSTM + path-gather + MLP + softmax).

Sharding: data-parallel over batch B=128 across 8 cores (16 samples/core).
All weights replicated; host concatenates per-core [BL*P, C] outputs.

Design (v2):
  - Gates padded+reordered [i, f, o, g] -> 8 groups of 128 partitions.
  - Recurrence PSUM: per dir one persistent [128, 2048] f32 ring viewed as
    [8 groups x 16 slots x 16 batch] (4 banks). Input projections (wih @ x)
    write pre-activations DIRECTLY into the ring half-window regions
    (start=True); per-step whh matmuls accumulate on top. No per-step PSUM
    preload, no pre-window SBUF buffer, no bias copies.
  - Biases: l0 via an all-ones column appended to the embedding table
    (emb col 200 = 1.0) + a bias row appended to the wih k1 chunk. Pad-gate
    biases (+-20) drive the pad rows of h to a known constant v, and l1's
    bias row is bias/v applied to the h-pad row.
  - x (embedded tokens), h0, h1 are SBUF-resident [128, 2*(NT+32)] bf16
    tiles (feature-chunk major, token minor); a zero step-slot at each end
    makes step 0 / step T-1 uniform (whh sees h=0).
  - All transposes (token-embedding staging, h1 export, MLP gather) go
    through the DMA crossbar (dma_start_transpose): zero PE/PSUM cost.
  - whh matmul order: g gates first, so tanh(g) runs while i/f/o matmuls
    finish; then sigmoid(i,f,o); cell ops on DVE; tanh(c) on Act.
"""

import numpy as np
import ml_dtypes

import concourse.bass as bass
import concourse.mybir as mybir
import concourse.tile as tile
from concourse import bacc
from concourse._compat import with_exitstack
from concourse.masks import make_identity

F32 = mybir.dt.float32
BF16 = mybir.dt.bfloat16
I32 = mybir.dt.int32
AF = mybir.ActivationFunctionType
BF16NP = ml_dtypes.bfloat16

# problem constants
V, E, H, T_FULL, B, PP, MLPD, C = 30000, 200, 200, 512, 128, 256, 200, 4
NCORES = 8
BL = B // NCORES          # 16 samples per core
GP = 8                    # padded gate groups (i0,i1,f0,f1,o0,o1,g0,g1)
KC = (128, 72)            # H contraction chunks
WIN = 8                   # steps per proj half-window
RING = 16                 # PSUM ring slots
DIRS = ("f", "b")
PAD_BIAS = 20.0           # pad-gate bias magnitude (drives h_pad to const v)


def _pad_v():
    """Device value of h at pad rows: sigma(20)*tanh(sigma(20)*tanh(20)), bf16-rounded."""
    s20 = np.float32(1.0 / (1.0 + np.exp(-np.float64(PAD_BIAS))))
    t20 = np.float32(np.tanh(np.float64(PAD_BIAS)))
    c = np.float32(s20 * t20)
    h = np.float32(s20 * np.float32(np.tanh(c)))
    return np.float32(BF16NP(h))


# ---------------------------------------------------------------- host packing

def _pack_gate_rows(w):
    """[800, ...] pytorch gate order (i,f,g,o) -> [1024, ...] order (i,f,o,g),
    each gate split into (128, 72+56pad) groups."""
    i, f, g, o = w[0:200], w[200:400], w[400:600], w[600:800]
    parts = []
    for gate in (i, f, o, g):
        parts.append(gate[0:128])
        pad = np.zeros((56,) + gate.shape[1:], np.float32)
        parts.append(np.concatenate([gate[128:200], pad], 0))
    return np.concatenate(parts, 0)


def prep_weights(inp):
    """Host-side packing of all weights. Returns dict of np arrays (shared by all cores)."""
    w = {}
    v = _pad_v()
    for layer in (0, 1):
        for d in DIRS:
            nm = f"l{layer}_{d}"
            wih = np.asarray(inp["wih_" + nm], np.float32)
            whh = np.asarray(inp["whh_" + nm], np.float32)
            bias = np.asarray(inp["bih_" + nm], np.float32) + np.asarray(inp["bhh_" + nm], np.float32)
            wihp = _pack_gate_rows(wih)                  # [1024, din]
            whhp = _pack_gate_rows(whh)                  # [1024, 200]
            bp = _pack_gate_rows(bias[:, None])[:, 0]    # [1024]
            if layer == 0:
                # pad-gate biases: drive pad rows of h to the constant v
                for k, val in ((1, PAD_BIAS), (3, -PAD_BIAS), (5, PAD_BIAS), (7, PAD_BIAS)):
                    bp[128 * k + 72:128 * (k + 1)] = val
            # tanh(x) = 2*sigmoid(2x) - 1: double the g-gate rows so the cell
            # can use a single sigmoid LUT pass over all four gates
            wihp[768:1024] *= 2.0
            whhp[768:1024] *= 2.0
            bp[768:1024] *= 2.0
            if layer == 0:
                wT = np.ascontiguousarray(wihp.T)        # [200, 1024]
                w[f"wih_{nm}_k0"] = wT[0:128].astype(BF16NP)
                w[f"wih_{nm}_k1"] = np.concatenate(
                    [wT[128:200], bp[None, :]], 0).astype(BF16NP)      # [73, 1024]
            else:
                wT = np.ascontiguousarray(wihp.T)        # [400, 1024]
                w[f"wih_{nm}_k0"] = wT[0:128].astype(BF16NP)
                w[f"wih_{nm}_k1"] = np.concatenate(
                    [wT[128:200], (bp / v)[None, :]], 0).astype(BF16NP)  # [73, 1024]
                w[f"wih_{nm}_k2"] = np.ascontiguousarray(wT[200:328]).astype(BF16NP)
                w[f"wih_{nm}_k3"] = np.ascontiguousarray(wT[328:400]).astype(BF16NP)
            whT = np.ascontiguousarray(whhp.T)           # [200, 1024]
            w[f"whh_{nm}_k0"] = whT[0:128].astype(BF16NP)
            w[f"whh_{nm}_k1"] = np.ascontiguousarray(whT[128:200]).astype(BF16NP)
    # embedding, padded to 256 cols with a ones column at 200
    emb = np.asarray(inp["emb"], np.float32)
    emb_p = np.zeros((V, 256), np.float32)
    emb_p[:, 0:200] = emb
    emb_p[:, 200] = 1.0
    w["emb_p"] = emb_p.astype(BF16NP)
    # MLP
    w1T = np.asarray(inp["w1"], np.float32).T            # [800, 200]
    w1Tp = np.concatenate([w1T[0:400], np.zeros((112, MLPD), np.float32),
                           w1T[400:800], np.zeros((112, MLPD), np.float32)], 0)  # [1024, 200]
    for ci in range(8):
        w[f"w1_k{ci}"] = w1Tp[128 * ci:128 * (ci + 1)].astype(BF16NP)
    b1 = np.asarray(inp["b1"], np.float32)
    b1p = np.zeros((128, 2), np.float32)
    b1p[:, 0] = b1[0:128]
    b1p[0:72, 1] = b1[128:200]
    w["b1"] = b1p
    w2T = np.asarray(inp["w2"], np.float32).T            # [200, 4]
    w["w2_k0"] = w2T[0:128].astype(BF16NP)
    w["w2_k1"] = np.ascontiguousarray(w2T[128:200]).astype(BF16NP)
    w["b2"] = np.tile(np.asarray(inp["b2"], np.float32)[None, :], (128, 1))
    return w


def prep_core_inputs(inp, wshared, core, T):
    """Per-core input map: shared weights + this core's token/path indices."""
    NT = T * BL
    b0 = core * BL
    tokens = np.asarray(inp["tokens"], np.int64)[:T, b0:b0 + BL]  # [T, BL]
    flat = tokens.reshape(NT).astype(np.int32)                    # t-major
    ntile = NT // 128
    m = dict(wshared)
    m["tok_idx"] = np.ascontiguousarray(flat.reshape(ntile, 128).T.astype(np.int32))
    paths = np.asarray(inp["paths"], np.int64)[b0:b0 + BL]        # [BL, P, 2]
    bcol = np.arange(BL, dtype=np.int64)[:, None, None]
    idx = np.where(paths >= 0, BL * paths + bcol, NT)             # invalid -> zero row
    nel = BL * PP
    ptile = nel // 128
    for k in range(2):
        fk = idx[:, :, k].reshape(nel).astype(np.int32)
        m[f"path_idx_k{k}"] = np.ascontiguousarray(fk.reshape(ptile, 128).T)
    return m


# ---------------------------------------------------------------- device kernel

GORDER = (6, 7, 0, 1, 2, 3, 4, 5)   # whh matmul group order: g gates first


@with_exitstack
def bilstm_kernel(ctx, tc, io, T):
    nc = tc.nc
    NT = T * BL
    NTP = NT + 32                     # +2 zero step-slots (front/back)
    NHW = T // WIN                    # proj half-windows per dir per layer
    nel = BL * PP

    const = ctx.enter_context(tc.tile_pool(name="const", bufs=1))

    # ---- load weights to SBUF. tok_idx first (gathers need it), then l0
    # weights (sync); everything else drip-fed into the l0 loop on the Act DGE.
    tok_idx = const.tile([128, NT // 128], I32)
    nc.sync.dma_start(tok_idx[:], io["tok_idx"][:])
    sb = {}
    deferred = []
    for layer in (0, 1):
        nkin = 2 if layer == 0 else 4
        for d in DIRS:
            nm = f"l{layer}_{d}"
            for ci in range(nkin):
                kn = (128, 73, 128, 72)[ci] if layer == 1 else (128, 73)[ci]
                t = const.tile([kn, 1024], BF16, tag=f"wih{nm}{ci}", name=f"wih{nm}{ci}")
                sb[f"wih_{nm}_k{ci}"] = t
                if layer == 0:
                    nc.sync.dma_start(t[:], io[f"wih_{nm}_k{ci}"][:])
                else:
                    deferred.append((t, f"wih_{nm}_k{ci}"))
            for ci in range(2):
                t = const.tile([KC[ci], 1024], BF16, tag=f"whh{nm}{ci}", name=f"whh{nm}{ci}")
                sb[f"whh_{nm}_k{ci}"] = t
                if layer == 0:
                    nc.sync.dma_start(t[:], io[f"whh_{nm}_k{ci}"][:])
                else:
                    deferred.append((t, f"whh_{nm}_k{ci}"))
    for ci in range(8):
        t = const.tile([128, MLPD], BF16, tag=f"w1{ci}", name=f"w1s{ci}")
        sb[f"w1_k{ci}"] = t
        deferred.append((t, f"w1_k{ci}"))
    for nm, shp, dt in (("b1", [128, 2], F32), ("w2_k0", [128, 4], BF16),
                        ("w2_k1", [72, 4], BF16), ("b2", [128, 4], F32)):
        t = const.tile(shp, dt, tag=nm, name=nm + "_s")
        sb[nm] = t
        deferred.append((t, nm))
    pidx = {}
    for k in range(2):
        pidx[k] = const.tile([128, nel // 128], I32, tag=f"pidx{k}", name=f"pidx{k}")
        deferred.append((pidx[k], f"path_idx_k{k}"))

    def pop_deferred(n=1):
        for _ in range(n):
            if deferred:
                t, nm = deferred.pop(0)
                nc.scalar.dma_start(t[:], io[nm][:])
    zrow = const.tile([128, 512], BF16, tag="zrow", name="zrow")
    nc.vector.memset(zrow[:], 0.0)
    ones32 = const.tile([128, 32], F32, tag="ones32", name="ones32")
    nc.vector.memset(ones32[:], 1.0)
    ident_bf = const.tile([128, 128], BF16, tag="identb", name="identb")
    make_identity(nc, ident_bf[:])

    # ---- persistent state
    big = ctx.enter_context(tc.tile_pool(name="big", bufs=1))
    h0 = {d: big.tile([128, 2 * NTP], BF16, tag=f"h0{d}", name=f"h0{d}") for d in DIRS}
    cst = {d: big.tile([128, 32], F32, tag=f"c{d}", name=f"c{d}") for d in DIRS}
    cell = ctx.enter_context(tc.tile_pool(name="cell", bufs=4))

    # ---- DRAM scratch for MLP gather (zero-fills drip-fed into the l0 loop)
    h1r = nc.dram_tensor("h1r", [NT + 1, 512], BF16, kind="Internal").ap()
    zf = [("z", blk) for blk in range(NT // 128)] + [("zrow", None)]

    def pop_zfill(n=1):
        for _ in range(n):
            if zf:
                kind, blk = zf.pop(0)
                if kind == "z":
                    nc.scalar.dma_start(h1r[128 * blk:128 * (blk + 1), 400:512],
                                        zrow[:, 0:112])
                else:
                    nc.scalar.dma_start(h1r[NT:NT + 1, :], zrow[0:1, :])

    def hcol(t):
        return 16 + BL * t            # column of step t inside a chunk region

    def zero_endslots(tl):
        for ci in range(2):
            nc.vector.memset(tl[:, ci * NTP:ci * NTP + 16], 0.0)
            nc.vector.memset(tl[:, ci * NTP + hcol(T):ci * NTP + hcol(T) + 16], 0.0)

    for d in DIRS:
        zero_endslots(h0[d])

    # ---------------- recurrence machinery
    # PSUM ring layout per dir: [half(2) x group(8) x slot(8) x batch(16)] f32.
    # Halves are bank-disjoint (2 banks each); exactly one start=True per bank
    # per window (first write of a restarted bank group overwrites, later
    # writes accumulate).
    GPERM = (0, 4, 1, 5, 2, 6, 3, 7)  # proj piece order: bank-starts first

    def whh_block(pr, layer, d, t):
        """16 accumulating matmuls adding whh @ h_{t-1} onto the ring slot of t."""
        nm = f"l{layer}_{d}"
        half = (t // WIN) % 2
        slot = t % WIN
        tp = t - 1 if d == "f" else t + 1
        src = h0[d] if layer == 0 else h1[d]
        n = 0
        for g in GORDER:
            col = half * 1024 + g * 128 + slot * BL
            for ci in range(2):
                cn = KC[ci]
                rhs = src[0:cn, ci * NTP + hcol(tp): ci * NTP + hcol(tp) + BL]
                n += 1
                nc.tensor.matmul(pr[:, col:col + BL],
                                 sb[f"whh_{nm}_k{ci}"][:, 128 * g:128 * (g + 1)],
                                 rhs, start=False, stop=(n == 16))

    def proj_piece(pr, layer, d, hw, g):
        """wih matmuls writing pre for (dir d, half-window hw, group g) into the ring."""
        nm = f"l{layer}_{d}"
        nkin = 2 if layer == 0 else 4
        col = (hw % 2) * 1024 + g * 128
        dst = pr[:, col:col + WIN * BL]               # [128, 128]
        for ci in range(nkin):
            if layer == 0:
                kn = (128, 73)[ci]
                rhs = xt[0:kn, ci * NT + 128 * hw: ci * NT + 128 * (hw + 1)]
            else:
                kn = (128, 73, 128, 72)[ci]
                srct = h0["f"] if ci < 2 else h0["b"]
                cc = ci % 2
                rhs = srct[0:kn, cc * NTP + hcol(WIN * hw): cc * NTP + hcol(WIN * hw) + 128]
            nc.tensor.matmul(dst, sb[f"wih_{nm}_k{ci}"][:, 128 * g:128 * (g + 1)],
                             rhs, start=(ci == 0 and g in (0, 4)), stop=False)

    def cell_step(pr, layer, d, t):
        """One sigmoid over all gates (g pre-doubled), then the c/h update.

        i' = s[0:32], f' = s[32:64], o' = s[64:96], sg2 = s[96:128] = sigmoid(2g)
        c = f'*c + i'*(2*sg2 - 1);  h = o'*tanh(c)
        """
        half = (t // WIN) % 2
        slot = t % WIN
        prv = pr[:].rearrange("p (g c) -> p g c", g=2 * GP)[
            :, 8 * half:8 * half + 8, slot * BL:(slot + 1) * BL]
        sg = cell.tile([128, 128], F32, tag=f"sg{d}", name=f"sg{d}")
        c1 = cell.tile([128, 32], F32, tag=f"c1{d}", name=f"c1{d}")
        a1 = cell.tile([128, 32], F32, tag=f"a1{d}", name=f"a1{d}")
        b1 = cell.tile([128, 32], F32, tag=f"b1{d}", name=f"b1{d}")
        tc_ = cell.tile([128, 32], F32, tag=f"tc{d}", name=f"tc{d}")
        nc.scalar.activation(sg[:].rearrange("p (g n) -> p g n", g=GP),
                             prv[:, 0:8, :], AF.Sigmoid)
        nc.vector.tensor_mul(c1[:], sg[:, 32:64], cst[d][:])
        # D = 2*sigmoid(2g) - 1 = tanh(g), then t1 = D*i', c = f'*c + t1
        nc.vector.scalar_tensor_tensor(a1[:], sg[:, 96:128], 2.0, ones32[:],
                                       mybir.AluOpType.mult, mybir.AluOpType.subtract)
        nc.vector.tensor_mul(b1[:], a1[:], sg[:, 0:32])
        nc.vector.tensor_add(cst[d][:], c1[:], b1[:])
        nc.scalar.activation(tc_[:], cst[d][:], AF.Tanh)
        dstt = h0[d] if layer == 0 else h1[d]
        hout = dstt[:, :].rearrange("p (c n) -> p c n", c=2)[:, :, hcol(t):hcol(t) + BL]
        nc.vector.tensor_mul(hout,
                             sg[:, 64:96].rearrange("p (c n) -> p c n", c=2),
                             tc_[:, :].rearrange("p (c n) -> p c n", c=2))

    # ---------------- recurrence: one PSUM pool spans both layers
    h1 = None
    with tc.tile_pool(name="psrec", bufs=1, space="PSUM") as prp:
        pr = {d: prp.tile([128, GP * RING * BL], F32, tag=f"pr{d}", name=f"pr{d}")
              for d in DIRS}

        # ---------------- layer 0 (with embedding gather interleaved)
        with tc.tile_pool(name="xtp", bufs=1) as xtp, \
             tc.tile_pool(name="gst", bufs=12) as gst:
            xt = xtp.tile([128, 2 * NT], BF16, tag="xt", name="xt")

            def emit_xt_tile(i):
                stage = gst.tile([128, 256], BF16, tag="xg", name="xg")
                nc.gpsimd.indirect_dma_start(
                    out=stage[:], out_offset=None, in_=io["emb_p"][:],
                    in_offset=bass.IndirectOffsetOnAxis(ap=tok_idx[:, i:i + 1], axis=0))
                for ci in range(2):
                    nc.sync.dma_start_transpose(
                        out=xt[:, ci * NT + 128 * i: ci * NT + 128 * (i + 1)],
                        in_=stage[:, 128 * ci:128 * (ci + 1)])

            xt_front = list(range(NHW))
            xt_back = list(range(NHW - 1, -1, -1))
            done = set()

            def pop_xt(lst):
                while lst:
                    i = lst.pop(0)
                    if i not in done:
                        done.add(i)
                        emit_xt_tile(i)
                        return

            for _ in range(10):
                pop_xt(xt_front)
                pop_xt(xt_back)
            for d in DIRS:
                nc.vector.memset(cst[d][:], 0.0)

            for hwp, dd in ((0, "f"), (NHW - 1, "b")):
                for g in GPERM:
                    proj_piece(pr[dd], 0, dd, hwp, g)
            for hw in range(NHW):
                for tau in range(WIN):
                    tf = WIN * hw + tau
                    tb = T - 1 - tf
                    if hw + 1 < NHW:
                        proj_piece(pr["f"], 0, "f", hw + 1, GPERM[tau])
                    whh_block(pr["f"], 0, "f", tf)
                    if NHW - 2 - hw >= 0:
                        proj_piece(pr["b"], 0, "b", NHW - 2 - hw, GPERM[tau])
                    whh_block(pr["b"], 0, "b", tb)
                    cell_step(pr["f"], 0, "f", tf)
                    cell_step(pr["b"], 0, "b", tb)
                    if tau == 2:
                        pop_xt(xt_front)
                    if tau == 6:
                        pop_xt(xt_back)
                    if tau == 0:
                        pop_deferred()
                    if tau in (3, 5):
                        pop_zfill()

        pop_deferred(len(deferred))
        pop_zfill(len(zf))

        # ---------------- layer 1
        with tc.tile_pool(name="h1p", bufs=1) as h1p, \
             tc.tile_pool(name="rowst", bufs=4) as rowp:
            h1 = {d: h1p.tile([128, 2 * NTP], BF16, tag=f"h1{d}", name=f"h1{d}")
                  for d in DIRS}
            for d in DIRS:
                zero_endslots(h1[d])
                nc.vector.memset(cst[d][:], 0.0)

            def export_h1(hw):
                """Transpose+store one 8-step block per dir to row-major h1r."""
                for d in DIRS:
                    blk = hw if d == "f" else NHW - 1 - hw
                    c0 = 0 if d == "f" else 200
                    stage = rowp.tile([128, 256], BF16, tag=f"rows{d}", name=f"rows{d}")
                    nc.sync.dma_start_transpose(
                        out=stage[:, 0:128],
                        in_=h1[d][0:128, hcol(WIN * blk):hcol(WIN * blk) + 128])
                    nc.sync.dma_start_transpose(
                        out=stage[:, 128:208],
                        in_=h1[d][0:80, NTP + hcol(WIN * blk):NTP + hcol(WIN * blk) + 128])
                    nc.sync.dma_start(h1r[128 * blk:128 * (blk + 1), c0:c0 + 200],
                                      stage[:, 0:200])

            for hwp, dd in ((0, "f"), (NHW - 1, "b")):
                for g in GPERM:
                    proj_piece(pr[dd], 1, dd, hwp, g)
            for hw in range(NHW):
                for tau in range(WIN):
                    tf = WIN * hw + tau
                    tb = T - 1 - tf
                    if hw + 1 < NHW:
                        proj_piece(pr["f"], 1, "f", hw + 1, GPERM[tau])
                    whh_block(pr["f"], 1, "f", tf)
                    if NHW - 2 - hw >= 0:
                        proj_piece(pr["b"], 1, "b", NHW - 2 - hw, GPERM[tau])
                    whh_block(pr["b"], 1, "b", tb)
                    cell_step(pr["f"], 1, "f", tf)
                    cell_step(pr["b"], 1, "b", tb)
                export_h1(hw)

    # ---------------- MLP + softmax (PE transposes: DMA xbar is too slow here)
    with tc.tile_pool(name="mlp", bufs=2) as mpool, \
         tc.tile_pool(name="gath", bufs=6) as gath, \
         tc.tile_pool(name="osm", bufs=4) as opool, \
         tc.tile_pool(name="psm", bufs=2, space="PSUM") as psm, \
         tc.tile_pool(name="pstp", bufs=4, space="PSUM") as pstp, \
         tc.tile_pool(name="ps2", bufs=2, space="PSUM") as ps2p:
        ECHUNK = 512
        nchunk = nel // ECHUNK
        for e in range(nchunk):
            mlpT = mpool.tile([128, 8 * ECHUNK], BF16, tag="mlpT", name="mlpT")
            for s in range(4):
                for k in range(2):
                    gt = gath.tile([128, 512], BF16, tag="g", name="gt")
                    nc.gpsimd.indirect_dma_start(
                        out=gt[:], out_offset=None, in_=h1r[:],
                        in_offset=bass.IndirectOffsetOnAxis(
                            ap=pidx[k][:, 4 * e + s:4 * e + s + 1], axis=0),
                        bounds_check=NT, oob_is_err=False)
                    for f4 in range(4):
                        pt = pstp.tile([128, 128], BF16, tag="tp", name="tpm")
                        nc.tensor.transpose(pt[:], gt[:, 128 * f4:128 * (f4 + 1)],
                                            ident_bf[:])
                        dstp = mlpT[:, ECHUNK * (4 * k + f4) + 128 * s:
                                    ECHUNK * (4 * k + f4) + 128 * (s + 1)]
                        if f4 % 2 == 0:
                            nc.vector.tensor_copy(dstp, pt[:])
                        else:
                            nc.scalar.copy(dstp, pt[:])
            hidT = mpool.tile([128, 2 * ECHUNK], BF16, tag="hidT", name="hidT")
            for m in range(2):
                pm = KC[m]
                psum = psm.tile([128, ECHUNK], F32, tag="proj", name="mm1ps")
                for kc in range(8):
                    nc.tensor.matmul(psum[:pm, :], sb[f"w1_k{kc}"][:, 128 * m:128 * m + pm],
                                     mlpT[:, ECHUNK * kc:ECHUNK * (kc + 1)],
                                     start=(kc == 0), stop=(kc == 7))
                nc.scalar.activation(hidT[:pm, ECHUNK * m:ECHUNK * m + ECHUNK], psum[:pm, :],
                                     AF.Tanh, bias=sb["b1"][:pm, m:m + 1])
            for s in range(4):
                ps2 = ps2p.tile([128, 4], F32, tag="mm2", name="mm2ps")
                for ci in range(2):
                    cn = KC[ci]
                    nc.tensor.matmul(ps2[:], hidT[:cn, ECHUNK * ci + 128 * s:
                                                  ECHUNK * ci + 128 * (s + 1)],
                                     sb[f"w2_k{ci}"][:], start=(ci == 0), stop=(ci == 1))
                lg = opool.tile([128, 4], F32, tag="lg", name="lg")
                ex = opool.tile([128, 4], F32, tag="ex", name="ex")
                sm = opool.tile([128, 1], F32, tag="sm", name="sm")
                rc = opool.tile([128, 1], F32, tag="rc", name="rc")
                ot = opool.tile([128, 4], F32, tag="ot", name="ot")
                nc.vector.tensor_add(lg[:], ps2[:], sb["b2"][:])
                nc.scalar.activation(ex[:], lg[:], AF.Exp)
                nc.vector.tensor_reduce(sm[:], ex[:], axis=mybir.AxisListType.X,
                                        op=mybir.AluOpType.add)
                nc.vector.reciprocal(rc[:], sm[:])
                nc.vector.tensor_scalar_mul(ot[:], ex[:], rc[:])
                nc.sync.dma_start(io["out"][ECHUNK * e + 128 * s: ECHUNK * e + 128 * (s + 1), :],
                                  ot[:])


# ---------------------------------------------------------------- build + run

def build(T=T_FULL, do_compile=True):
    nc = bacc.Bacc("TRN2", target_bir_lowering=False, debug=False)
    NT = T * BL
    nel = BL * PP
    io = {}

    def din(name, shape, dtype):
        io[name] = nc.dram_tensor(name, list(shape), dtype, kind="ExternalInput").ap()

    din("emb_p", (V, 256), BF16)
    din("tok_idx", (128, NT // 128), I32)
    for k in range(2):
        din(f"path_idx_k{k}", (128, nel // 128), I32)
    for layer in (0, 1):
        nkin = 2 if layer == 0 else 4
        kshape = (128, 73) if layer == 0 else (128, 73, 128, 72)
        for d in DIRS:
            nm = f"l{layer}_{d}"
            for ci in range(nkin):
                din(f"wih_{nm}_k{ci}", (kshape[ci], 1024), BF16)
            for ci in range(2):
                din(f"whh_{nm}_k{ci}", (KC[ci], 1024), BF16)
    for ci in range(8):
        din(f"w1_k{ci}", (128, MLPD), BF16)
    din("b1", (128, 2), F32)
    din("w2_k0", (128, 4), BF16)
    din("w2_k1", (72, 4), BF16)
    din("b2", (128, 4), F32)
    io["out"] = nc.dram_tensor("out", [nel, C], F32, kind="ExternalOutput").ap()

    with tile.TileContext(nc) as tc:
        bilstm_kernel(tc, io, T)
    if do_compile:
        nc.compile()
    return nc


_CACHED = {}


def kernel(**inputs):
    T = np.asarray(inputs["tokens"]).shape[0]
    if T not in _CACHED:
        _CACHED[T] = build(T)
    nc = _CACHED[T]
    wshared = prep_weights(inputs)
    in_maps = [prep_core_inputs(inputs, wshared, core, T) for core in range(NCORES)]
    from concourse.bass_utils import run_bass_kernel_spmd
    res = run_bass_kernel_spmd(nc, in_maps, core_ids=list(range(NCORES)))
    return np.concatenate([res.results[i]["out"] for i in range(NCORES)], 0)
